# revision 5
# baseline (speedup 1.0000x reference)
"""Causal self-attention (B=4, T=2048, C=1024, 16 heads) on 8 trn2 NeuronCores.

Sharding: core c handles batch b = c//2 and an 8-head half hh = c%2
(tensor parallel over heads). Each core computes its heads' attention
output projected through its slice of w_proj rows; the host sums the two
partial projections per batch.

Device-side layout (per core):
  - QKV^T orientation: Q^T/K^T [feat, T] come straight out of the QKV
    matmul (lhsT = w chunk, rhs = x^T), V comes out in [T, feat] via the
    swapped orientation (lhsT = x^T chunk, rhs = w_v).
  - Scores are computed transposed, S^T[k, q], so softmax sums ride the
    A@V matmul as a ones-column appended to V (M=65).
  - exp has no max-subtraction (logits are N(0,1)-ish, |s|<40 -> safe in
    fp32), computed by ACT with the 1/sqrt(D) fused into its scale imm.
  - A and V' are float32r so the A@V matmul runs at full rate (N=512)
    with ~11-bit mantissa precision; QKV in bf16; proj in f32r.
"""
import sys

if "/opt/trn_rl_repo" not in sys.path:
    sys.path.insert(0, "/opt/trn_rl_repo")

import numpy as np
import ml_dtypes

B, T, C = 4, 2048, 1024
NH, D = 16, 64
P = 128
QC = 512           # q-chunk width
NQC = T // QC      # 4
NKB = T // P       # 16 k-blocks
GS = 3             # k-blocks per exp group (3 PSUM banks)
DH = 512           # per-core head feature width (8 heads * 64)

_CACHE = {}


def _build():
    import concourse.mybir as mybir
    import concourse.tile as tile
    from concourse import bacc

    f32 = mybir.dt.float32
    f32r = mybir.dt.float32r
    bf16 = mybir.dt.bfloat16
    MULT = mybir.AluOpType.mult
    EXP = mybir.ActivationFunctionType.Exp

    nc = bacc.Bacc(None, target_bir_lowering=False, debug=False)

    xt_d = nc.declare_dram_parameter("xt", [C, T], bf16, isOutput=False)
    wqk_d = nc.declare_dram_parameter("wqk", [C, 2 * DH], bf16, isOutput=False)
    wv_d = nc.declare_dram_parameter("wv", [C, DH], bf16, isOutput=False)
    wp_d = nc.declare_dram_parameter("wp", [DH, C], f32r, isOutput=False)
    dm_d = nc.declare_dram_parameter("dmask", [P, 4 * QC], f32, isOutput=False)
    out_d = nc.declare_dram_parameter("outT", [C, T], f32, isOutput=True)

    NCC = C // P  # 8 contraction chunks for QKV

    with tile.TileContext(nc) as tc:
        with (
            tc.tile_pool(name="pconst", bufs=1) as pconst,
            tc.tile_pool(name="pw", bufs=1) as pw,
            tc.tile_pool(name="px", bufs=2) as px,
            tc.tile_pool(name="pq", bufs=2) as pq,
            tc.tile_pool(name="pk", bufs=1) as pk,
            tc.tile_pool(name="pv", bufs=1) as pv,
            tc.tile_pool(name="pa", bufs=2) as pa,
            tc.tile_pool(name="psml", bufs=2) as psml,
            tc.tile_pool(name="posb", bufs=1) as posb,
            tc.tile_pool(name="psS", bufs=2, space="PSUM") as psS,
            tc.tile_pool(name="psX", bufs=2, space="PSUM") as psX,
        ):
            # ---- constants / weights ----
            dm_t = pconst.tile([P, 4 * QC], f32, name="dm")
            nc.sync.dma_start(dm_t[:], dm_d[:])
            ones_c = pconst.tile([P, 8, 1], f32, name="ones_c")
            nc.vector.memset(ones_c[:], 1.0)
            ones_f = pconst.tile([P, P], f32, name="ones_f")
            nc.vector.memset(ones_f[:], 1.0)
            ones_r = pconst.tile([P, P], f32r, name="ones_r")
            nc.vector.tensor_copy(ones_r[64:65, :], ones_f[64:65, :])

            wqk_t = []
            wv_t = []
            for i in range(NCC):
                t_ = pw.tile([P, 2 * DH], bf16, tag=f"wqk{i}", name=f"wqk{i}")
                nc.sync.dma_start(t_[:], wqk_d[P * i : P * (i + 1), :])
                wqk_t.append(t_)
                v_ = pw.tile([P, DH], bf16, tag=f"wv{i}", name=f"wv{i}")
                nc.sync.dma_start(v_[:], wv_d[P * i : P * (i + 1), :])
                wv_t.append(v_)
            wp_t = []
            for i in range(4):
                t_ = pw.tile([P, C], f32r, tag=f"wp{i}", name=f"wp{i}")
                nc.sync.dma_start(t_[:], wp_d[P * i : P * (i + 1), :])
                wp_t.append(t_)

            # ---- persistent stores ----
            # K^T bf16 per (j-block, qc): [128 feat, 512 k-cols]
            k_sb = [
                [pk.tile([P, QC], bf16, tag=f"k{j}_{m}", name=f"k{j}_{m}")
                 for m in range(NQC)]
                for j in range(4)
            ]
            # V' f32r per k-block: [128 k, 8 heads, 65] (col 64 = ones)
            vp = [pv.tile([P, 8, 65], f32r, tag=f"vp{kb}", name=f"vp{kb}")
                  for kb in range(NKB)]
            # O^T f32r per (cin-chunk, qc): [128 feat, 512 q]
            o_sb = [
                [posb.tile([P, QC], f32r, tag=f"o{i}_{m}", name=f"o{i}_{m}")
                 for m in range(NQC)]
                for i in range(4)
            ]

            for n in range(NQC):
                # ---- QKV for q-chunk n ----
                x_t = []
                for i in range(NCC):
                    t_ = px.tile([P, QC], bf16, tag=f"x{i}", name=f"x{i}_{n}")
                    nc.sync.dma_start(
                        t_[:], xt_d[P * i : P * (i + 1), QC * n : QC * (n + 1)]
                    )
                    x_t.append(t_)

                q_t = []
                for j in range(8):
                    pp = psX.tile([P, QC], f32, tag="pp", name=f"qkps{j}_{n}")
                    for i in range(NCC):
                        nc.tensor.matmul(
                            pp[:],
                            wqk_t[i][:, P * j : P * (j + 1)],
                            x_t[i][:],
                            start=(i == 0),
                            stop=(i == NCC - 1),
                        )
                    if j < 4:
                        qt = pq.tile([P, QC], bf16, tag=f"q{j}", name=f"q{j}_{n}")
                        nc.vector.tensor_copy(qt[:], pp[:])
                        q_t.append(qt)
                    else:
                        nc.vector.tensor_copy(k_sb[j - 4][n][:], pp[:])

                for qb in range(4):
                    kb = 4 * n + qb
                    pp = psX.tile([P, QC], f32, tag="pp", name=f"vps{kb}")
                    for i in range(NCC):
                        nc.tensor.matmul(
                            pp[:],
                            x_t[i][:, P * qb : P * (qb + 1)],
                            wv_t[i][:],
                            start=(i == 0),
                            stop=(i == NCC - 1),
                        )
                    nc.vector.tensor_copy(vp[kb][:, :, 64:65], ones_c[:])
                    nc.vector.tensor_copy(
                        vp[kb][:, :, 0:64],
                        pp[:].rearrange("p (h d) -> p h d", d=64),
                    )

                # ---- attention for all heads at q-chunk n ----
                nkb = 4 * (n + 1)
                groups = [
                    list(range(g, min(g + GS, nkb))) for g in range(0, nkb, GS)
                ]
                for h in range(8):
                    jq, half = divmod(h, 2)
                    r0 = 64 * half
                    rhs_q = q_t[jq][r0 : r0 + 64, :]
                    oacc = psml.tile([P, QC], f32, tag="oacc", bufs=3,
                                     name=f"oacc{h}_{n}")
                    for gi, grp in enumerate(groups):
                        w = QC * len(grp)
                        sg = psS.tile([P, GS * QC], f32, tag="sg",
                                      name=f"sg{h}_{n}_{gi}")
                        for t_i, kb in enumerate(grp):
                            lhs_k = k_sb[jq][kb // 4][
                                r0 : r0 + 64, P * (kb % 4) : P * (kb % 4 + 1)
                            ]
                            nc.tensor.matmul(
                                sg[:, QC * t_i : QC * (t_i + 1)],
                                lhs_k,
                                rhs_q,
                                start=True,
                                stop=True,
                            )
                        at = pa.tile([P, GS * QC], f32r, tag="at",
                                     name=f"at{h}_{n}_{gi}")
                        nc.scalar.activation(at[:, :w], sg[:, :w], EXP, scale=0.125)
                        for t_i, kb in enumerate(grp):
                            if kb >= 4 * n:
                                tt = kb - 4 * n
                                nc.vector.tensor_tensor(
                                    at[:, QC * t_i : QC * (t_i + 1)],
                                    at[:, QC * t_i : QC * (t_i + 1)],
                                    dm_t[:, QC * tt : QC * (tt + 1)],
                                    MULT,
                                )
                        pp = psX.tile([P, QC], f32, tag="pp",
                                      name=f"avps{h}_{n}_{gi}")
                        for t_i, kb in enumerate(grp):
                            nc.tensor.matmul(
                                pp[0:65, :],
                                vp[kb][:, h, :],
                                at[:, QC * t_i : QC * (t_i + 1)],
                                start=(t_i == 0),
                                stop=(t_i == len(grp) - 1),
                            )
                        if gi == 0:
                            nc.vector.tensor_copy(oacc[0:65, :], pp[0:65, :])
                        else:
                            nc.vector.tensor_add(
                                oacc[0:65, :], oacc[0:65, :], pp[0:65, :]
                            )
                    sums_r = psml.tile([P, QC], f32r, tag="rb",
                                       name=f"sums{h}_{n}")
                    nc.vector.tensor_copy(sums_r[64:65, :], oacc[64:65, :])
                    sps = psX.tile([P, QC], f32, tag="pp", name=f"sps{h}_{n}")
                    nc.tensor.matmul(sps[:], ones_r[64:65, :], sums_r[64:65, :],
                                     start=True, stop=True)
                    rr = psml.tile([P, QC], f32, tag="rr", name=f"rr{h}_{n}")
                    nc.vector.reciprocal_approx_fast(rr[:], sps[:])
                    if h % 2 == 0:
                        nc.vector.tensor_tensor(
                            o_sb[h // 2][n][0:64, :],
                            oacc[0:64, :],
                            rr[0:64, :],
                            MULT,
                        )
                    else:
                        onorm = psml.tile([P, QC], f32r, tag="onorm",
                                          name=f"onorm{h}_{n}")
                        nc.vector.tensor_tensor(
                            onorm[0:64, :],
                            oacc[0:64, :],
                            rr[0:64, :],
                            MULT,
                        )
                        nc.sync.dma_start(
                            o_sb[h // 2][n][64:128, :], onorm[0:64, :]
                        )

            # ---- output projection ----
            for j2 in range(8):
                for n in range(NQC):
                    pp = psX.tile([P, QC], f32, tag="pp", name=f"pj{j2}_{n}")
                    for i2 in range(4):
                        nc.tensor.matmul(
                            pp[:],
                            wp_t[i2][:, P * j2 : P * (j2 + 1)],
                            o_sb[i2][n][:],
                            start=(i2 == 0),
                            stop=(i2 == 3),
                        )
                    oo = psml.tile([P, QC], f32, tag="oo", name=f"oo{j2}_{n}")
                    nc.vector.tensor_copy(oo[:], pp[:])
                    nc.sync.dma_start(
                        out_d[P * j2 : P * (j2 + 1), QC * n : QC * (n + 1)],
                        oo[:],
                    )

    nc.compile()
    return nc


def _get_nc():
    if "nc" not in _CACHE:
        _CACHE["nc"] = _build()
    return _CACHE["nc"]


def _make_dmask():
    dm = np.zeros((P, 4 * QC), np.float32)
    for t in range(4):
        for r in range(P):
            dm[r, QC * t + 128 * t + r : QC * (t + 1)] = 1.0
    return dm


def _in_maps(x, w_qkv, w_proj):
    bf = ml_dtypes.bfloat16
    dm = _make_dmask()
    maps = []
    for c in range(8):
        b, hh = divmod(c, 2)
        xT = np.ascontiguousarray(x[b].T).astype(bf)
        qcols = w_qkv[:, DH * hh : DH * hh + DH]
        kcols = w_qkv[:, C + DH * hh : C + DH * hh + DH]
        vcols = w_qkv[:, 2 * C + DH * hh : 2 * C + DH * hh + DH]
        maps.append({
            "xt": xT,
            "wqk": np.concatenate([qcols, kcols], axis=1).astype(bf),
            "wv": np.ascontiguousarray(vcols).astype(bf),
            "wp": np.ascontiguousarray(w_proj[DH * hh : DH * hh + DH, :],
                                       dtype=np.float32),
            "dmask": dm,
        })
    return maps


def _run(x, w_qkv, w_proj, trace=False):
    from concourse.bass_utils import run_bass_kernel_spmd

    nc = _get_nc()
    maps = _in_maps(x, w_qkv, w_proj)
    res = run_bass_kernel_spmd(nc, maps, list(range(8)), trace=trace)
    out = np.empty((B, T, C), np.float32)
    for b in range(B):
        out[b] = res.results[2 * b]["outT"].T + res.results[2 * b + 1]["outT"].T
    return out, res


def kernel(**inputs):
    x = np.asarray(inputs["x"], dtype=np.float32)
    w_qkv = np.asarray(inputs["w_qkv"], dtype=np.float32)
    w_proj = np.asarray(inputs["w_proj"], dtype=np.float32)
    out, _ = _run(x, w_qkv, w_proj, trace=False)
    return out


def kernel_traced(**inputs):
    x = np.asarray(inputs["x"], dtype=np.float32)
    w_qkv = np.asarray(inputs["w_qkv"], dtype=np.float32)
    w_proj = np.asarray(inputs["w_proj"], dtype=np.float32)
    out, res = _run(x, w_qkv, w_proj, trace=True)
    return out, res
